# revision 16
# baseline (speedup 1.0000x reference)
"""Correlation layer (FlowNet-style) Trainium2 Bass kernel, v2.

Problem: in1, in2: [8, 256, 128, 128] fp32.
out[b, 9*dy+dx, y, x] = mean_c in1[b,c,y,x] * in2pad[b,c,y+dy,x+dx],
in2 zero-padded by 4 per spatial side, dy,dx in [0,9).  Output
[8, 81, 128, 128] fp32.  Data-parallel over batch: 1 batch / core.

Host prep (free): in1 scaled by 1/256 (folds the channel mean),
(x-outer, y-inner) tile layout, bf16; in2 zero-padded to 136x136, bf16.
Output produced in bf16 and upcast on host.

Per-core pipeline (all bf16 matmul operands, fp32 PSUM accumulate):

1. Correlation matmuls.  Pixel tile = 32 y  x 4 x (128 pixels); each of
   the 4 x-columns is an independent col-tiled matmul (tile_position
   (0,32g)): stationary = in1[c, 32 pixels of column g], moving = its
   own 9-wide window in2pad[c, y0:y0+40, xg:xg+9] (N=360).  The four
   groups run concurrently on the PE array quarters, so a tile costs
   ~2x360 cycles for 128 pixels instead of 2x480 with a shared window,
   and PSUM comes out as [128, 40, 9] with IDENTICAL free layout for
   every partition: pixel (g,u) has channel ch at free offset 9u + ch.

2. Evacuate psum -> SBUF in one full-width copy per tile (cast to
   bf16).  No window-compaction instructions needed.

3. Sheared dump.  DMA the [40,9] blocks to a DRAM scratch where chunk
   of pixel p' (global raster index y*128+x) starts at byte offset
   2*(369*p' - 9u).  The -9u per-partition shear is absorbed by the
   flat DRAM stride (u-stride 369*128-9): each pixel's 81 useful
   channel values land EXACTLY at [369*p', 369*p'+81), and chunks
   never overlap (gap 9 between x-neighbours, exact tiling in y).

4. XBAR transpose read-back: dma_start(transpose=True) with source AP
   [[369, 2048], [1, 128]] reads each pixel's 81 channels (+47 junk
   cols) and transposes to SBUF [128ch, 2048pix] -- already in final
   [channel, raster(y,x)] order.

5. Store rows 0..81 to out (bf16), host upcasts to fp32.
"""

import numpy as np
import ml_dtypes
from contextlib import ExitStack

import concourse.bacc as bacc
import concourse.tile as tile
import concourse.mybir as mybir
import concourse.bass as bass
from concourse import bass_utils

# ---- problem constants (hardcoded per contract) ----
B = 8
C = 256
H = W = 128
PAD = 4
D = 9            # displacements per axis
CH = D * D       # 81 output channels
HP = WP = H + 2 * PAD   # 136 padded

YB = 32          # y rows per pixel tile
XBW = 4          # x cols per pixel tile (one col-tiled matmul each)
MV_Y = YB + 8    # moving window rows per group (40)
N_YB = H // YB   # 4
N_XB = W // XBW  # 32
NG = 4           # col-tile groups per tile
TBATCH = 16      # tiles buffered per dump batch (half a yb row)

BLK = MV_Y * D   # 360 elems per pixel chunk
NPIX = H * W     # 16384
SCR_ELEMS = BLK * NPIX   # pitch-360 scratch: chunks tile exactly

XH = 2048        # pixels per xbar batch (16 y rows)
N_XBATCH = NPIX // XH  # 8

BF16 = mybir.dt.bfloat16
FP32 = mybir.dt.float32


def prep_in_maps(in1: np.ndarray, in2: np.ndarray) -> list[dict]:
    """Host-side prep: scale+layout in1, pad in2, cast bf16."""
    in1 = np.asarray(in1, dtype=np.float32)
    in2 = np.asarray(in2, dtype=np.float32)
    assert in1.shape == (B, C, H, W) and in2.shape == (B, C, H, W)
    # [B, cb, c, yb, x, y],  scaled by 1/C (folds the channel mean)
    a = (in1 * (1.0 / C)).reshape(B, 2, 128, N_YB, YB, W)
    a = np.ascontiguousarray(a.transpose(0, 1, 2, 3, 5, 4)).astype(
        ml_dtypes.bfloat16
    )
    # [B, cb, c, 136, 136] zero-padded
    p = np.pad(in2, ((0, 0), (0, 0), (PAD, PAD), (PAD, PAD))).reshape(
        B, 2, 128, HP, WP
    ).astype(ml_dtypes.bfloat16)
    return [{"in1": a[b], "in2": p[b]} for b in range(B)]


def build_nc():
    nc = bacc.Bacc("TRN2", target_bir_lowering=False, debug=False)
    in1_d = nc.dram_tensor(
        "in1", [2, 128, N_YB, W, YB], BF16, kind="ExternalInput"
    ).ap()
    in2_d = nc.dram_tensor(
        "in2", [2, 128, HP, WP], BF16, kind="ExternalInput"
    ).ap()
    out_d = nc.dram_tensor("out", [CH, H, W], BF16, kind="ExternalOutput").ap()
    # one scratch tensor per yb: DRAM deps are tracked per-tensor, so a
    # shared scratch would serialize yb+1's dumps behind yb's xbar reads
    scr_t = [
        nc.dram_tensor(f"scr{yb}", [SCR_ELEMS // N_YB], BF16, kind="Internal")
        for yb in range(N_YB)
    ]

    with tile.TileContext(nc) as tc, ExitStack() as es:
        in2_pool = es.enter_context(tc.tile_pool(name="in2p", bufs=1))
        in1_pool = es.enter_context(tc.tile_pool(name="in1c", bufs=1))
        wv_pool = es.enter_context(tc.tile_pool(name="wv", bufs=4))
        xb_pool = es.enter_context(tc.tile_pool(name="xb", bufs=4))
        psum_pool = es.enter_context(tc.tile_pool(name="ps", bufs=8, space="PSUM"))

        # ---- inputs split over both HWDGE queues (in1 on sync, in2 on
        # scalar) in first-use order: the first matmul waits only for
        # ~2.4 MB and the two streams load in parallel. ----
        in2p = in2_pool.tile([128, 2, HP, WP], BF16, tag="in2p")
        in1c = in1_pool.tile([128, 2, N_YB, W, YB], BF16, tag="in1c")
        row_chunks = [(0, 40), (40, 72), (72, 104), (104, HP)]
        for yb in range(N_YB):
            r0, r1 = row_chunks[yb]
            for cb in range(2):
                nc.sync.dma_start(
                    in1c[:, cb, yb, :, :], in1_d[cb, :, yb, :, :]
                )
                nc.scalar.dma_start(
                    in2p[:, cb, r0:r1, :], in2_d[cb, :, r0:r1, :]
                )

        for yb in range(N_YB):
            y0 = yb * YB
            for half in range(2):
                xbase = half * TBATCH
                wv = wv_pool.tile([128, TBATCH, MV_Y, D], BF16, tag="wv")
                for t in range(TBATCH):
                    xb = xbase + t
                    ps = psum_pool.tile([128, MV_Y, D], FP32, tag="ps")
                    for cb in range(2):
                        for g in range(NG):
                            # group g = x-column 32g + xb (strided!):
                            # partition 32g+u <-> pixel (x=32g+xb, y)
                            # so each partition's scratch chunks are
                            # x-consecutive -> 11.5 KB dump descriptors
                            xg = 32 * g + xb
                            stat = in1c[:, cb, yb, xg, :]
                            mov = in2p[:, cb, y0:y0 + MV_Y, xg:xg + D]
                            nc.tensor.matmul(
                                ps[32 * g:32 * (g + 1), :, :],
                                stat,
                                mov,
                                start=(cb == 0),
                                stop=(cb == 1),
                                tile_position=(0, 32 * g),
                            )
                    dst = wv[:, t, :, :]
                    if xb % 2 == 0:
                        nc.vector.tensor_copy(dst, ps[:, :, :])
                    else:
                        nc.scalar.copy(dst, ps[:, :, :])

                # sheared dump (gpsimd SWDGE queue): chunk of pixel p'
                # (yb-local raster) at 360*p' - 9u; useful 81 land at
                # 360*p'.  Chunk overlaps are junk-over-junk ->
                # order-free.
                for g in range(NG):
                    src = wv[32 * g:32 * (g + 1), :, :, :].rearrange(
                        "p t a b -> p (t a b)"
                    )
                    base = BLK * (32 * g + xbase)
                    dst = bass.AP(
                        scr_t[yb],
                        base,
                        [[BLK * W - D, 32], [1, BLK * TBATCH]],
                    )
                    nc.gpsimd.dma_start(dst, src)

            # ---- read back via xbar transpose + store, all on the sync
            # queue: both xbars issue before either store so the store's
            # in-queue wait never delays the second xbar ----
            xbts = []
            for k in (2 * yb, 2 * yb + 1):
                xbt = xb_pool.tile([128, XH], BF16, tag="xbt")
                src = bass.AP(
                    scr_t[yb], BLK * XH * (k % 2), [[BLK, XH], [1, 128]]
                )
                nc.sync.dma_start(xbt[:, :], src, transpose=True)
                xbts.append(xbt)
            for k, xbt in zip((2 * yb, 2 * yb + 1), xbts):
                store = out_d[:, 16 * k:16 * (k + 1), :].rearrange(
                    "c a b -> c (a b)"
                )
                nc.sync.dma_start(store, xbt[0:CH, :])

    nc.compile()
    return nc


_NC_CACHE = None


def _get_nc():
    global _NC_CACHE
    if _NC_CACHE is None:
        _NC_CACHE = build_nc()
    return _NC_CACHE


def kernel(in1: np.ndarray, in2: np.ndarray) -> np.ndarray:
    nc = _get_nc()
    in_maps = prep_in_maps(in1, in2)
    res = bass_utils.run_bass_kernel_spmd(nc, in_maps, core_ids=list(range(B)))
    return np.stack(
        [res.results[b]["out"].astype(np.float32) for b in range(B)], axis=0
    )


# revision 18
# speedup vs baseline: 1.1133x; 1.1133x over previous
"""Correlation layer (FlowNet-style) Trainium2 Bass kernel, v2.

Problem: in1, in2: [8, 256, 128, 128] fp32.
out[b, 9*dy+dx, y, x] = mean_c in1[b,c,y,x] * in2pad[b,c,y+dy,x+dx],
in2 zero-padded by 4 per spatial side, dy,dx in [0,9).  Output
[8, 81, 128, 128] fp32.  Data-parallel over batch: 1 batch / core.

Host prep (free): in1 scaled by 1/256 (folds the channel mean),
(x-outer, y-inner) tile layout, bf16; in2 zero-padded to 136x136, bf16.
Output produced in bf16 and upcast on host.

Per-core pipeline (all bf16 matmul operands, fp32 PSUM accumulate):

1. Correlation matmuls.  Pixel tile = 32 y  x 4 x (128 pixels); each of
   the 4 x-columns is an independent col-tiled matmul (tile_position
   (0,32g)): stationary = in1[c, 32 pixels of column g], moving = its
   own 9-wide window in2pad[c, y0:y0+40, xg:xg+9] (N=360).  The four
   groups run concurrently on the PE array quarters, so a tile costs
   ~2x360 cycles for 128 pixels instead of 2x480 with a shared window,
   and PSUM comes out as [128, 40, 9] with IDENTICAL free layout for
   every partition: pixel (g,u) has channel ch at free offset 9u + ch.

2. Evacuate psum -> SBUF in one full-width copy per tile (cast to
   bf16).  No window-compaction instructions needed.

3. Sheared dump.  DMA the [40,9] blocks to a DRAM scratch where chunk
   of pixel p' (global raster index y*128+x) starts at byte offset
   2*(369*p' - 9u).  The -9u per-partition shear is absorbed by the
   flat DRAM stride (u-stride 369*128-9): each pixel's 81 useful
   channel values land EXACTLY at [369*p', 369*p'+81), and chunks
   never overlap (gap 9 between x-neighbours, exact tiling in y).

4. XBAR transpose read-back: dma_start(transpose=True) with source AP
   [[369, 2048], [1, 128]] reads each pixel's 81 channels (+47 junk
   cols) and transposes to SBUF [128ch, 2048pix] -- already in final
   [channel, raster(y,x)] order.

5. Store rows 0..81 to out (bf16), host upcasts to fp32.
"""

import numpy as np
import ml_dtypes
from contextlib import ExitStack

import concourse.bacc as bacc
import concourse.tile as tile
import concourse.mybir as mybir
import concourse.bass as bass
from concourse import bass_utils

# ---- problem constants (hardcoded per contract) ----
B = 8
C = 256
H = W = 128
PAD = 4
D = 9            # displacements per axis
CH = D * D       # 81 output channels
HP = WP = H + 2 * PAD   # 136 padded

YB = 32          # y rows per pixel tile
XBW = 4          # x cols per pixel tile (one col-tiled matmul each)
MV_Y = YB + 8    # moving window rows per group (40)
N_YB = H // YB   # 4
N_XB = W // XBW  # 32
NG = 4           # col-tile groups per tile
TBATCH = 16      # tiles buffered per dump batch (half a yb row)

BLK = MV_Y * D   # 360 elems per pixel chunk
NPIX = H * W     # 16384
SCR_ELEMS = BLK * NPIX   # pitch-360 scratch: chunks tile exactly

XH = 2048        # pixels per xbar batch (16 y rows)
N_XBATCH = NPIX // XH  # 8

BF16 = mybir.dt.bfloat16
FP32 = mybir.dt.float32


def prep_in_maps(in1: np.ndarray, in2: np.ndarray) -> list[dict]:
    """Host-side prep: scale+layout in1, pad in2, cast bf16."""
    in1 = np.asarray(in1, dtype=np.float32)
    in2 = np.asarray(in2, dtype=np.float32)
    assert in1.shape == (B, C, H, W) and in2.shape == (B, C, H, W)
    # [B, cb, c, yb, x, y],  scaled by 1/C (folds the channel mean)
    a = (in1 * (1.0 / C)).reshape(B, 2, 128, N_YB, YB, W)
    a = np.ascontiguousarray(a.transpose(0, 1, 2, 3, 5, 4)).astype(
        ml_dtypes.bfloat16
    )
    # [B, cb, c, 136, 136] zero-padded
    p = np.pad(in2, ((0, 0), (0, 0), (PAD, PAD), (PAD, PAD))).reshape(
        B, 2, 128, HP, WP
    ).astype(ml_dtypes.bfloat16)
    return [{"in1": a[b], "in2": p[b]} for b in range(B)]


def build_nc():
    nc = bacc.Bacc("TRN2", target_bir_lowering=False, debug=False)
    in1_d = nc.dram_tensor(
        "in1", [2, 128, N_YB, W, YB], BF16, kind="ExternalInput"
    ).ap()
    in2_d = nc.dram_tensor(
        "in2", [2, 128, HP, WP], BF16, kind="ExternalInput"
    ).ap()
    out_d = nc.dram_tensor("out", [CH, H, W], BF16, kind="ExternalOutput").ap()
    # one scratch tensor per yb: DRAM deps are tracked per-tensor, so a
    # shared scratch would serialize yb+1's dumps behind yb's xbar reads
    scr_t = [
        nc.dram_tensor(f"scr{yb}", [SCR_ELEMS // N_YB], BF16, kind="Internal")
        for yb in range(N_YB)
    ]

    with tile.TileContext(nc) as tc, ExitStack() as es:
        in2_pool = es.enter_context(tc.tile_pool(name="in2p", bufs=1))
        in1_pool = es.enter_context(tc.tile_pool(name="in1c", bufs=1))
        wv_pool = es.enter_context(tc.tile_pool(name="wv", bufs=3))
        xb_pool = es.enter_context(tc.tile_pool(name="xb", bufs=2))
        psum_pool = es.enter_context(tc.tile_pool(name="ps", bufs=8, space="PSUM"))

        # ---- inputs split over both HWDGE queues (in1 on sync, in2 on
        # scalar) in first-use order: the first matmul waits only for
        # ~2.4 MB and the two streams load in parallel. ----
        in2p = in2_pool.tile([128, 2, HP, WP], BF16, tag="in2p")
        in1c = in1_pool.tile([128, 2, N_YB, W, YB], BF16, tag="in1c")
        row_chunks = [(0, 40), (40, 72), (72, 104), (104, HP)]
        for yb in range(N_YB):
            r0, r1 = row_chunks[yb]
            for cb in range(2):
                nc.sync.dma_start(
                    in1c[:, cb, yb, :, :], in1_d[cb, :, yb, :, :]
                )
                nc.scalar.dma_start(
                    in2p[:, cb, r0:r1, :], in2_d[cb, :, r0:r1, :]
                )

        for yb in range(N_YB):
            y0 = yb * YB
            for half in range(2):
                xbase = half * TBATCH
                wv = wv_pool.tile([128, TBATCH, MV_Y, D], BF16, tag="wv")
                for t in range(TBATCH):
                    xb = xbase + t
                    ps = psum_pool.tile([128, MV_Y, D], FP32, tag="ps")
                    for cb in range(2):
                        for g in range(NG):
                            # group g = x-column 32g + xb (strided!):
                            # partition 32g+u <-> pixel (x=32g+xb, y)
                            # so each partition's scratch chunks are
                            # x-consecutive -> 11.5 KB dump descriptors
                            xg = 32 * g + xb
                            stat = in1c[:, cb, yb, xg, :]
                            mov = in2p[:, cb, y0:y0 + MV_Y, xg:xg + D]
                            nc.tensor.matmul(
                                ps[32 * g:32 * (g + 1), :, :],
                                stat,
                                mov,
                                start=(cb == 0),
                                stop=(cb == 1),
                                tile_position=(0, 32 * g),
                            )
                    dst = wv[:, t, :, :]
                    if xb % 2 == 0:
                        nc.vector.tensor_copy(dst, ps[:, :, :])
                    else:
                        nc.scalar.copy(dst, ps[:, :, :])

                # sheared dump (gpsimd SWDGE queue): chunk of pixel p'
                # (yb-local raster) at 360*p' - 9u; useful 81 land at
                # 360*p'.  Chunk overlaps are junk-over-junk ->
                # order-free.
                for g in range(NG):
                    src = wv[32 * g:32 * (g + 1), :, :, :].rearrange(
                        "p t a b -> p (t a b)"
                    )
                    base = BLK * (32 * g + xbase)
                    dst = bass.AP(
                        scr_t[yb],
                        base,
                        [[BLK * W - D, 32], [1, BLK * TBATCH]],
                    )
                    nc.gpsimd.dma_start(dst, src)

            # ---- read back via xbar transpose (sync queue) + store on
            # the scalar HWDGE queue (input loads there are long done) ----
            for k in (2 * yb, 2 * yb + 1):
                xbt = xb_pool.tile([128, XH], BF16, tag="xbt")
                src = bass.AP(
                    scr_t[yb], BLK * XH * (k % 2), [[BLK, XH], [1, 128]]
                )
                nc.sync.dma_start(xbt[:, :], src, transpose=True)
                store = out_d[:, 16 * k:16 * (k + 1), :].rearrange(
                    "c a b -> c (a b)"
                )
                nc.scalar.dma_start(store, xbt[0:CH, :])

    nc.compile()
    return nc


_NC_CACHE = None


def _get_nc():
    global _NC_CACHE
    if _NC_CACHE is None:
        _NC_CACHE = build_nc()
    return _NC_CACHE


def kernel(in1: np.ndarray, in2: np.ndarray) -> np.ndarray:
    nc = _get_nc()
    in_maps = prep_in_maps(in1, in2)
    res = bass_utils.run_bass_kernel_spmd(nc, in_maps, core_ids=list(range(B)))
    return np.stack(
        [res.results[b]["out"].astype(np.float32) for b in range(B)], axis=0
    )
